# revision 12
# baseline (speedup 1.0000x reference)
"""MAMDense Trainium2 kernel.

C[m, n] = max_k(x[m,k] * W[n,k]) + min_k(x[m,k] * W[n,k]) + bias[n]
x: [2048, 1024] f32, W: [1024, 1024] f32, bias: [1024] f32 -> C: [2048, 1024] f32

Strategy (data parallel over M, 8 cores, 256 rows each):
- x rows on SBUF partitions (2 tiles of 128 rows x 1024 K).
- Weight rows stored one-per-partition in SBUF ([128, 8*1024]); for each
  output column n, the PE broadcasts W[n, :] across all 128 partitions via a
  ones-vector outer product into PSUM.
- One fused custom DVE instruction per (m-tile, n) computes
  running_max(x*w) + running_min(x*w) + bias[n] over K in a single pass;
  the output AP has free-stride 0 so the final (k=K-1) value - the answer -
  lands directly in C[:, n]. No separate reduce or extract instructions.
- C tiles DMA straight out in natural [M, N] layout.
"""
import numpy as np

M, K, N = 2048, 1024, 1024
N_CORES = 8
M_LOC = M // N_CORES  # 256
P = 128
FMAX = 3.4028234663852886e38

_STATE = {}


def _register_mam_op():
    """Register the fused multiply->scan(max)+scan(min)+bias DVE op."""
    import concourse.dve_ops as dve_ops
    from concourse.dve_ops import DveOp
    from concourse.dve_spec import (
        Spec, Src0, Src1, C0, C1, scan, AluOp, lower, _has_src1,
    )
    from concourse.dve_uop import DveOpSpec

    name = "MAM_BIAS_FUSED_ANT"
    for op in dve_ops.OPS:
        if op.name == name:
            return op

    prod = Src0 * Src1

    def _ref(in0, in1, s0, s1, imm2):
        pr = in0 * in1
        return (np.maximum.accumulate(pr, axis=-1)
                + np.minimum.accumulate(pr, axis=-1) + s1)

    spec = Spec(
        body=scan(AluOp.MAX, prod) + scan(AluOp.MIN, prod, init=C0) + C1,
        reference=_ref,
    )
    shas = {}
    for ver in ("v3", "v4"):
        try:
            s = DveOpSpec(name=name, opcode=1, uops=lower(spec, ver=ver),
                          rd1_en=_has_src1(spec))
            shas[ver] = s.sha(ver)
        except Exception:
            pass
    op = DveOp(name, spec, subdim=False, uops_sha=shas)
    dve_ops.OPS.append(op)
    dve_ops._SUB_OPCODE_FOR_NAME[name] = (
        dve_ops._CUSTOM_DVE_ROW_BASE + len(dve_ops.OPS) - 1
    )
    dve_ops.CUSTOM_DVE_SPECS[name] = spec
    return op


def build_nc(replicas: int = 1):
    """Build + compile the per-core Bacc program. `replicas` repeats the
    compute body (for timing-by-differencing in test harnesses)."""
    import concourse.bacc as bacc
    import concourse.mybir as mybir
    from concourse.tile import TileContext

    MAM = _register_mam_op()

    nc = bacc.Bacc("TRN2", target_bir_lowering=False, debug=False)
    x = nc.dram_tensor("x", [M_LOC, K], mybir.dt.float32, kind="ExternalInput")
    w = nc.dram_tensor("weight", [N, K], mybir.dt.float32, kind="ExternalInput")
    b = nc.dram_tensor("bias", [N], mybir.dt.float32, kind="ExternalInput")
    ident = nc.dram_tensor("ident", [P, P], mybir.dt.float32,
                           kind="ExternalInput")
    out = nc.dram_tensor("out", [M_LOC, N], mybir.dt.float32,
                         kind="ExternalOutput")

    NT = M_LOC // P  # 2 m-tiles

    with TileContext(nc) as tc:
        with tc.tile_pool(name="const", bufs=1) as cpool, \
             tc.tile_pool(name="psum", bufs=2, space="PSUM") as ppool:
            # --- loads -----------------------------------------------------
            xt = []
            for t in range(NT):
                xti = cpool.tile([P, K], mybir.dt.float32, name=f"xt{t}",
                                 tag=f"xt{t}")
                nc.sync.dma_start(out=xti[:], in_=x.ap()[t * P:(t + 1) * P, :])
                xt.append(xti)
            bias_t = cpool.tile([1, N], mybir.dt.float32, tag="bias_t")
            nc.sync.dma_start(out=bias_t[:], in_=b.ap()[None, :])
            ones = cpool.tile([1, P], mybir.dt.float32, tag="ones")
            nc.gpsimd.memset(ones[:], 1.0)
            ident_t = cpool.tile([P, P], mybir.dt.float32, tag="ident_t")
            nc.sync.dma_start(out=ident_t[:], in_=ident.ap()[:, :])
            wblk = []
            for blk in range(N // P):
                wb_ = cpool.tile([P, K], mybir.dt.float32, name=f"wblk{blk}",
                                 tag=f"wblk{blk}")
                nc.sync.dma_start(out=wb_[:],
                                  in_=w.ap()[blk * P:(blk + 1) * P, :])
                wblk.append(wb_)

            # bias broadcast across partitions: ones^T @ bias_row -> PSUM
            bias_bc = cpool.tile([P, N], mybir.dt.float32, tag="bias_bc")
            pb0 = ppool.tile([P, N], mybir.dt.float32, tag="pb")
            for h in range(N // 512):
                nc.tensor.matmul(pb0[:, h * 512:(h + 1) * 512], ones[:],
                                 bias_t[0:1, h * 512:(h + 1) * 512],
                                 start=True, stop=True)
            nc.scalar.copy(bias_bc[:], pb0[:])

            ct = [cpool.tile([P, N], mybir.dt.float32, name=f"ct{t}",
                             tag=f"ct{t}")
                  for t in range(NT)]

            # --- main loop ---------------------------------------------------
            # Row n of W is broadcast across all 128 partitions in two PE
            # steps (no DMA involved):
            #   A: pe_row[1, K]  = e_{n%128}.T @ Wblk[n//128]   (row extract)
            #      ACT copies pe_row PSUM -> SBUF (st_row).
            #   B: wb[128, K]    = ones.T @ st_row              (broadcast)
            with tc.tile_pool(name="stage", bufs=3) as spool:
                for _ in range(replicas):
                    for n in range(N):
                        blk, row = divmod(n, P)
                        sel = ident_t[:, row:row + 1]          # [128, 1]
                        pe_row = ppool.tile([1, K], mybir.dt.float32,
                                            tag="pe_row")
                        for h in range(K // 512):
                            nc.tensor.matmul(
                                pe_row[0:1, h * 512:(h + 1) * 512], sel,
                                wblk[blk][:, h * 512:(h + 1) * 512],
                                start=True, stop=True)
                        st_row = spool.tile([1, K], mybir.dt.float32,
                                            tag="st_row")
                        nc.scalar.copy(st_row[:], pe_row[:])
                        pb = ppool.tile([P, K], mybir.dt.float32, tag="pb")
                        for h in range(K // 512):
                            nc.tensor.matmul(
                                pb[:, h * 512:(h + 1) * 512], ones[:],
                                st_row[0:1, h * 512:(h + 1) * 512],
                                start=True, stop=True)
                        for t in range(NT):
                            nc.vector._custom_dve(
                                MAM,
                                out=ct[t][:, n:n + 1].broadcast_to([P, K]),
                                in0=xt[t][:],
                                in1=pb[:],
                                s0=FMAX,
                                s1=bias_bc[:, n:n + 1],
                            )

            # --- store -------------------------------------------------------
            for t in range(NT):
                nc.sync.dma_start(out=out.ap()[t * P:(t + 1) * P, :],
                                  in_=ct[t][:])
    nc.compile()
    return nc


def _get_runner(replicas: int = 1):
    key = ("runner", replicas)
    if key not in _STATE:
        import jax
        import numpy as _np
        from jax.sharding import Mesh, PartitionSpec
        from jax.experimental.shard_map import shard_map
        import concourse.mybir as mybir
        from concourse import bass2jax
        from concourse.bass2jax import _bass_exec_p, install_neuronx_cc_hook

        install_neuronx_cc_hook()
        nc = build_nc(replicas)

        partition_name = (nc.partition_id_tensor.name
                          if nc.partition_id_tensor else None)
        in_names, out_names, out_avals, zero_shapes = [], [], [], []
        for alloc in nc.m.functions[0].allocations:
            if not isinstance(alloc, mybir.MemoryLocationSet):
                continue
            nm = alloc.memorylocations[0].name
            if alloc.kind == "ExternalInput":
                if nm != partition_name:
                    in_names.append(nm)
            elif alloc.kind == "ExternalOutput":
                out_names.append(nm)
                shape = tuple(alloc.tensor_shape)
                dtype = mybir.dt.np(alloc.dtype)
                out_avals.append(jax.core.ShapedArray(shape, dtype))
                zero_shapes.append((shape, dtype))
        all_in_names = list(in_names) + out_names
        if partition_name is not None:
            all_in_names.append(partition_name)

        def _body(*args):
            operands = list(args)
            if partition_name is not None:
                operands.append(bass2jax.partition_id_tensor())
            outs = _bass_exec_p.bind(
                *operands,
                out_avals=tuple(out_avals),
                in_names=tuple(all_in_names),
                out_names=tuple(out_names),
                lowering_input_output_aliases=(),
                sim_require_finite=True,
                sim_require_nnan=True,
                nc=nc,
            )
            return tuple(outs)

        devices = jax.devices()[:N_CORES]
        mesh = Mesh(_np.asarray(devices), ("core",))
        n_io = len(in_names) + len(out_names)
        fn = jax.jit(
            shard_map(_body, mesh=mesh,
                      in_specs=(PartitionSpec("core"),) * n_io,
                      out_specs=(PartitionSpec("core"),) * len(out_names),
                      check_rep=False),
            keep_unused=True,
        )
        _STATE[key] = (fn, in_names, out_names, out_avals, zero_shapes, mesh)
    return _STATE[key]


def _prepare(x, weight, bias, replicas=1):
    """device_put sharded inputs; returns a zero-arg callable running the
    kernel on device plus metadata for decoding outputs."""
    import jax
    from jax.sharding import NamedSharding, PartitionSpec
    fn, in_names, out_names, out_avals, zero_shapes, mesh = _get_runner(replicas)
    eye = np.eye(P, dtype=np.float32)
    per_core = {
        "x": [x[c * M_LOC:(c + 1) * M_LOC] for c in range(N_CORES)],
        "weight": [weight] * N_CORES,
        "bias": [bias] * N_CORES,
        "ident": [eye] * N_CORES,
    }
    concat_in = [np.concatenate(per_core[nm], axis=0) for nm in in_names]
    concat_zeros = [np.zeros((N_CORES * s[0], *s[1:]), d)
                    for (s, d) in zero_shapes]
    sharding = NamedSharding(mesh, PartitionSpec("core"))
    dev_in = [jax.device_put(a, sharding) for a in concat_in]
    dev_zero = [jax.device_put(a, sharding) for a in concat_zeros]

    def run():
        outs = fn(*dev_in, *dev_zero)
        jax.block_until_ready(outs)
        return outs

    return run, out_names, out_avals


def _run_sharded(x, weight, bias, replicas=1):
    run, out_names, out_avals = _prepare(x, weight, bias, replicas)
    return run(), out_names, out_avals


def kernel(x: np.ndarray, weight: np.ndarray, bias: np.ndarray) -> np.ndarray:
    x = np.ascontiguousarray(np.asarray(x, dtype=np.float32))
    weight = np.ascontiguousarray(np.asarray(weight, dtype=np.float32))
    bias = np.ascontiguousarray(np.asarray(bias, dtype=np.float32))
    outs, out_names, out_avals = _run_sharded(x, weight, bias, replicas=1)
    i = out_names.index("out")
    full = np.asarray(outs[i]).reshape(N_CORES * M_LOC, N)
    return full


# revision 13
# speedup vs baseline: 1.4522x; 1.4522x over previous
"""MAMDense Trainium2 kernel.

C[m, n] = max_k(x[m,k] * W[n,k]) + min_k(x[m,k] * W[n,k]) + bias[n]
x: [2048, 1024] f32, W: [1024, 1024] f32, bias: [1024] f32 -> C: [2048, 1024] f32

Strategy (data parallel over M, 8 cores, 256 rows each):
- x rows on SBUF partitions (2 tiles of 128 rows x 1024 K).
- Weight rows stored one-per-partition in SBUF ([128, 8*1024]); for each
  output column n, the PE broadcasts W[n, :] across all 128 partitions via a
  ones-vector outer product into PSUM.
- One fused custom DVE instruction per (m-tile, n) computes
  running_max(x*w) + running_min(x*w) + bias[n] over K in a single pass;
  the output AP has free-stride 0 so the final (k=K-1) value - the answer -
  lands directly in C[:, n]. No separate reduce or extract instructions.
- C tiles DMA straight out in natural [M, N] layout.
"""
import numpy as np

M, K, N = 2048, 1024, 1024
N_CORES = 8
M_LOC = M // N_CORES  # 256
P = 128
FMAX = 3.4028234663852886e38

_STATE = {}


def _register_mam_op():
    """Register the fused multiply->scan(max)+scan(min)+bias DVE op."""
    import concourse.dve_ops as dve_ops
    from concourse.dve_ops import DveOp
    from concourse.dve_spec import (
        Spec, Src0, Src1, C0, C1, scan, AluOp, lower, _has_src1,
    )
    from concourse.dve_uop import DveOpSpec

    name = "MAM_BIAS_FUSED_ANT"
    for op in dve_ops.OPS:
        if op.name == name:
            return op

    prod = Src0 * Src1

    def _ref(in0, in1, s0, s1, imm2):
        pr = in0 * in1
        return (np.maximum.accumulate(pr, axis=-1)
                + np.minimum.accumulate(pr, axis=-1) + s1)

    spec = Spec(
        body=scan(AluOp.MAX, prod) + scan(AluOp.MIN, prod, init=C0) + C1,
        reference=_ref,
    )
    shas = {}
    for ver in ("v3", "v4"):
        try:
            s = DveOpSpec(name=name, opcode=1, uops=lower(spec, ver=ver),
                          rd1_en=_has_src1(spec))
            shas[ver] = s.sha(ver)
        except Exception:
            pass
    op = DveOp(name, spec, subdim=False, uops_sha=shas)
    dve_ops.OPS.append(op)
    dve_ops._SUB_OPCODE_FOR_NAME[name] = (
        dve_ops._CUSTOM_DVE_ROW_BASE + len(dve_ops.OPS) - 1
    )
    dve_ops.CUSTOM_DVE_SPECS[name] = spec
    return op


def build_nc(replicas: int = 1):
    """Build + compile the per-core Bacc program. `replicas` repeats the
    compute body (for timing-by-differencing in test harnesses)."""
    import concourse.bacc as bacc
    import concourse.mybir as mybir
    from concourse.tile import TileContext

    MAM = _register_mam_op()

    nc = bacc.Bacc("TRN2", target_bir_lowering=False, debug=False)
    x = nc.dram_tensor("x", [M_LOC, K], mybir.dt.float32, kind="ExternalInput")
    w = nc.dram_tensor("weight", [N, K], mybir.dt.float32, kind="ExternalInput")
    b = nc.dram_tensor("bias", [N], mybir.dt.float32, kind="ExternalInput")
    ident = nc.dram_tensor("ident", [P, P], mybir.dt.float32,
                           kind="ExternalInput")
    out = nc.dram_tensor("out", [M_LOC, N], mybir.dt.float32,
                         kind="ExternalOutput")

    NT = M_LOC // P  # 2 m-tiles

    with TileContext(nc) as tc:
        with tc.tile_pool(name="const", bufs=1) as cpool, \
             tc.tile_pool(name="psum", bufs=2, space="PSUM") as ppool:
            # --- loads -----------------------------------------------------
            xt = []
            for t in range(NT):
                xti = cpool.tile([P, K], mybir.dt.float32, name=f"xt{t}",
                                 tag=f"xt{t}")
                nc.sync.dma_start(out=xti[:], in_=x.ap()[t * P:(t + 1) * P, :])
                xt.append(xti)
            bias_t = cpool.tile([1, N], mybir.dt.float32, tag="bias_t")
            nc.sync.dma_start(out=bias_t[:], in_=b.ap()[None, :])
            ones = cpool.tile([1, P], mybir.dt.float32, tag="ones")
            nc.gpsimd.memset(ones[:], 1.0)
            ident_t = cpool.tile([P, P], mybir.dt.float32, tag="ident_t")
            nc.sync.dma_start(out=ident_t[:], in_=ident.ap()[:, :])
            wblk = []
            for blk in range(N // P):
                wb_ = cpool.tile([P, K], mybir.dt.float32, name=f"wblk{blk}",
                                 tag=f"wblk{blk}")
                nc.sync.dma_start(out=wb_[:],
                                  in_=w.ap()[blk * P:(blk + 1) * P, :])
                wblk.append(wb_)

            # bias broadcast across partitions: ones^T @ bias_row -> PSUM
            bias_bc = cpool.tile([P, N], mybir.dt.float32, tag="bias_bc")
            pb0 = ppool.tile([P, N], mybir.dt.float32, tag="pb")
            for h in range(N // 512):
                nc.tensor.matmul(pb0[:, h * 512:(h + 1) * 512], ones[:],
                                 bias_t[0:1, h * 512:(h + 1) * 512],
                                 start=True, stop=True)
            nc.scalar.copy(bias_bc[:], pb0[:])

            ct = [cpool.tile([P, N], mybir.dt.float32, name=f"ct{t}",
                             tag=f"ct{t}")
                  for t in range(NT)]

            # --- main loop ---------------------------------------------------
            # Row n of W is broadcast across all 128 partitions in two PE
            # steps (no DMA involved):
            #   A: pe_row[1, K]  = e_{n%128}.T @ Wblk[n//128]   (row extract)
            #      ACT copies pe_row PSUM -> SBUF (st_row).
            #   B: wb[128, K]    = ones.T @ st_row              (broadcast)
            with tc.tile_pool(name="stage", bufs=6) as spool:
                for _ in range(replicas):
                    for n in range(N):
                        blk, row = divmod(n, P)
                        sel = ident_t[:, row:row + 1]          # [128, 1]
                        pe_row = ppool.tile([1, K], mybir.dt.float32,
                                            tag="pe_row")
                        for h in range(K // 512):
                            nc.tensor.matmul(
                                pe_row[0:1, h * 512:(h + 1) * 512], sel,
                                wblk[blk][:, h * 512:(h + 1) * 512],
                                start=True, stop=True)
                        st_row = spool.tile([1, K], mybir.dt.float32,
                                            tag="st_row")
                        nc.scalar.copy(st_row[:], pe_row[:])
                        pb = ppool.tile([P, K], mybir.dt.float32, tag="pb")
                        for h in range(K // 512):
                            nc.tensor.matmul(
                                pb[:, h * 512:(h + 1) * 512], ones[:],
                                st_row[0:1, h * 512:(h + 1) * 512],
                                start=True, stop=True)
                        for t in range(NT):
                            nc.vector._custom_dve(
                                MAM,
                                out=ct[t][:, n:n + 1].broadcast_to([P, K]),
                                in0=xt[t][:],
                                in1=pb[:],
                                s0=FMAX,
                                s1=bias_bc[:, n:n + 1],
                            )

            # --- store -------------------------------------------------------
            for t in range(NT):
                nc.sync.dma_start(out=out.ap()[t * P:(t + 1) * P, :],
                                  in_=ct[t][:])
    nc.compile()
    return nc


def _get_runner(replicas: int = 1):
    key = ("runner", replicas)
    if key not in _STATE:
        import jax
        import numpy as _np
        from jax.sharding import Mesh, PartitionSpec
        from jax.experimental.shard_map import shard_map
        import concourse.mybir as mybir
        from concourse import bass2jax
        from concourse.bass2jax import _bass_exec_p, install_neuronx_cc_hook

        install_neuronx_cc_hook()
        nc = build_nc(replicas)

        partition_name = (nc.partition_id_tensor.name
                          if nc.partition_id_tensor else None)
        in_names, out_names, out_avals, zero_shapes = [], [], [], []
        for alloc in nc.m.functions[0].allocations:
            if not isinstance(alloc, mybir.MemoryLocationSet):
                continue
            nm = alloc.memorylocations[0].name
            if alloc.kind == "ExternalInput":
                if nm != partition_name:
                    in_names.append(nm)
            elif alloc.kind == "ExternalOutput":
                out_names.append(nm)
                shape = tuple(alloc.tensor_shape)
                dtype = mybir.dt.np(alloc.dtype)
                out_avals.append(jax.core.ShapedArray(shape, dtype))
                zero_shapes.append((shape, dtype))
        all_in_names = list(in_names) + out_names
        if partition_name is not None:
            all_in_names.append(partition_name)

        def _body(*args):
            operands = list(args)
            if partition_name is not None:
                operands.append(bass2jax.partition_id_tensor())
            outs = _bass_exec_p.bind(
                *operands,
                out_avals=tuple(out_avals),
                in_names=tuple(all_in_names),
                out_names=tuple(out_names),
                lowering_input_output_aliases=(),
                sim_require_finite=True,
                sim_require_nnan=True,
                nc=nc,
            )
            return tuple(outs)

        devices = jax.devices()[:N_CORES]
        mesh = Mesh(_np.asarray(devices), ("core",))
        n_io = len(in_names) + len(out_names)
        fn = jax.jit(
            shard_map(_body, mesh=mesh,
                      in_specs=(PartitionSpec("core"),) * n_io,
                      out_specs=(PartitionSpec("core"),) * len(out_names),
                      check_rep=False),
            keep_unused=True,
        )
        _STATE[key] = (fn, in_names, out_names, out_avals, zero_shapes, mesh)
    return _STATE[key]


def _prepare(x, weight, bias, replicas=1):
    """device_put sharded inputs; returns a zero-arg callable running the
    kernel on device plus metadata for decoding outputs."""
    import jax
    from jax.sharding import NamedSharding, PartitionSpec
    fn, in_names, out_names, out_avals, zero_shapes, mesh = _get_runner(replicas)
    eye = np.eye(P, dtype=np.float32)
    per_core = {
        "x": [x[c * M_LOC:(c + 1) * M_LOC] for c in range(N_CORES)],
        "weight": [weight] * N_CORES,
        "bias": [bias] * N_CORES,
        "ident": [eye] * N_CORES,
    }
    concat_in = [np.concatenate(per_core[nm], axis=0) for nm in in_names]
    concat_zeros = [np.zeros((N_CORES * s[0], *s[1:]), d)
                    for (s, d) in zero_shapes]
    sharding = NamedSharding(mesh, PartitionSpec("core"))
    dev_in = [jax.device_put(a, sharding) for a in concat_in]
    dev_zero = [jax.device_put(a, sharding) for a in concat_zeros]

    def run():
        outs = fn(*dev_in, *dev_zero)
        jax.block_until_ready(outs)
        return outs

    return run, out_names, out_avals


def _run_sharded(x, weight, bias, replicas=1):
    run, out_names, out_avals = _prepare(x, weight, bias, replicas)
    return run(), out_names, out_avals


def kernel(x: np.ndarray, weight: np.ndarray, bias: np.ndarray) -> np.ndarray:
    x = np.ascontiguousarray(np.asarray(x, dtype=np.float32))
    weight = np.ascontiguousarray(np.asarray(weight, dtype=np.float32))
    bias = np.ascontiguousarray(np.asarray(bias, dtype=np.float32))
    outs, out_names, out_avals = _run_sharded(x, weight, bias, replicas=1)
    i = out_names.index("out")
    full = np.asarray(outs[i]).reshape(N_CORES * M_LOC, N)
    return full


# revision 14
# speedup vs baseline: 2.5590x; 1.7622x over previous
"""MAMDense Trainium2 kernel.

C[m, n] = max_k(x[m,k] * W[n,k]) + min_k(x[m,k] * W[n,k]) + bias[n]
x: [2048, 1024] f32, W: [1024, 1024] f32, bias: [1024] f32 -> C: [2048, 1024] f32

Strategy (data parallel over M, 8 cores, 256 rows each):
- W blocks live naturally in SBUF ([128 n-rows, K] x 8 blocks) — the custom
  DVE op reads them directly, no W movement at all.
- Each x row m is broadcast across all 128 partitions via PE (row-extract
  matmul into a PSUM quadrant, ACT copy to SBUF, ones-matmul broadcast).
  Rows are extracted 4 at a time into PSUM partition quadrants {0,32,64,96}
  (PE tile_position), so one ACT copy serves 4 rows.
- One fused custom DVE instruction per (m, n-block) computes
  running_max(w*x) + running_min(w*x) + bias over K in a single pass; the
  output AP has free-stride 0 so the final (k=K-1) value - the answer -
  lands directly in C^T[n-block][:, m]. No reduce/extract instructions.
- C^T tiles are PE-transposed back to natural [M, N] layout at the end.
"""
import numpy as np

M, K, N = 2048, 1024, 1024
N_CORES = 8
M_LOC = M // N_CORES  # 256
P = 128
NB = N // P           # 8 n-blocks
NT = M_LOC // P       # 2 m-tiles
FMAX = 3.4028234663852886e38

_STATE = {}


def _register_mam_op():
    """Register the fused multiply->scan(max)+scan(min)+bias DVE op."""
    import concourse.dve_ops as dve_ops
    from concourse.dve_ops import DveOp
    from concourse.dve_spec import (
        Spec, Src0, Src1, C0, C1, scan, AluOp, lower, _has_src1,
    )
    from concourse.dve_uop import DveOpSpec

    name = "MAM_BIAS_FUSED_ANT"
    for op in dve_ops.OPS:
        if op.name == name:
            return op

    prod = Src0 * Src1

    def _ref(in0, in1, s0, s1, imm2):
        pr = in0 * in1
        return (np.maximum.accumulate(pr, axis=-1)
                + np.minimum.accumulate(pr, axis=-1) + s1)

    spec = Spec(
        body=scan(AluOp.MAX, prod) + scan(AluOp.MIN, prod, init=C0) + C1,
        reference=_ref,
    )
    shas = {}
    for ver in ("v3", "v4"):
        try:
            s = DveOpSpec(name=name, opcode=1, uops=lower(spec, ver=ver),
                          rd1_en=_has_src1(spec))
            shas[ver] = s.sha(ver)
        except Exception:
            pass
    op = DveOp(name, spec, subdim=False, uops_sha=shas)
    dve_ops.OPS.append(op)
    dve_ops._SUB_OPCODE_FOR_NAME[name] = (
        dve_ops._CUSTOM_DVE_ROW_BASE + len(dve_ops.OPS) - 1
    )
    dve_ops.CUSTOM_DVE_SPECS[name] = spec
    return op


def build_nc(replicas: int = 1):
    """Build + compile the per-core Bacc program. `replicas` repeats the
    compute body (for timing-by-differencing in test harnesses)."""
    import concourse.bacc as bacc
    import concourse.mybir as mybir
    from concourse.tile import TileContext

    MAM = _register_mam_op()

    nc = bacc.Bacc("TRN2", target_bir_lowering=False, debug=False)
    x = nc.dram_tensor("x", [M_LOC, K], mybir.dt.float32, kind="ExternalInput")
    w = nc.dram_tensor("weight", [N, K], mybir.dt.float32, kind="ExternalInput")
    b = nc.dram_tensor("bias", [P, NB], mybir.dt.float32, kind="ExternalInput")
    ident = nc.dram_tensor("ident", [P, P], mybir.dt.float32,
                           kind="ExternalInput")
    out = nc.dram_tensor("out", [M_LOC, N], mybir.dt.float32,
                         kind="ExternalOutput")

    with TileContext(nc) as tc:
        with tc.tile_pool(name="const", bufs=1) as cpool, \
             tc.tile_pool(name="psum", bufs=2, space="PSUM") as ppool, \
             tc.tile_pool(name="stage", bufs=3) as spool:
            # --- loads -----------------------------------------------------
            xt = []
            for t in range(NT):
                xti = cpool.tile([P, K], mybir.dt.float32, name=f"xt{t}",
                                 tag=f"xt{t}")
                nc.sync.dma_start(out=xti[:], in_=x.ap()[t * P:(t + 1) * P, :])
                xt.append(xti)
            wblk = []
            for blk in range(NB):
                wb_ = cpool.tile([P, K], mybir.dt.float32, name=f"wblk{blk}",
                                 tag=f"wblk{blk}")
                nc.sync.dma_start(out=wb_[:],
                                  in_=w.ap()[blk * P:(blk + 1) * P, :])
                wblk.append(wb_)
            bias_pb = cpool.tile([P, NB], mybir.dt.float32, tag="bias_pb")
            nc.sync.dma_start(out=bias_pb[:], in_=b.ap()[:, :])
            ident_t = cpool.tile([P, P], mybir.dt.float32, tag="ident_t")
            nc.sync.dma_start(out=ident_t[:], in_=ident.ap()[:, :])
            ones_all = cpool.tile([P, P], mybir.dt.float32, tag="ones_all")
            nc.gpsimd.memset(ones_all[:], 1.0)

            ctT = [cpool.tile([P, M_LOC], mybir.dt.float32, name=f"ctT{blk}",
                              tag=f"ctT{blk}") for blk in range(NB)]
            ct = [cpool.tile([P, N], mybir.dt.float32, name=f"ct{t}",
                             tag=f"ct{t}") for t in range(NT)]

            # --- main loop: groups of 4 consecutive x rows -------------------
            for _ in range(replicas):
                for g in range(M_LOC // 4):
                    pe4 = ppool.tile([P, K], mybir.dt.float32, tag="pe4")
                    for j in range(4):
                        m = 4 * g + j
                        t, r = divmod(m, P)
                        q = 32 * j
                        sel = ident_t[:, r:r + 1]
                        for h in range(K // 512):
                            nc.tensor.matmul(
                                pe4[q:q + 1, h * 512:(h + 1) * 512], sel,
                                xt[t][:, h * 512:(h + 1) * 512],
                                start=True, stop=True, tile_position=(0, q))
                    st4 = spool.tile([P, K], mybir.dt.float32, tag="st4")
                    nc.scalar.copy(st4[:], pe4[:])
                    for j in range(4):
                        m = 4 * g + j
                        q = 32 * j
                        xps = ppool.tile([P, K], mybir.dt.float32, tag="xps")
                        for h in range(K // 512):
                            nc.tensor.matmul(
                                xps[:, h * 512:(h + 1) * 512],
                                ones_all[q:q + 1, :],
                                st4[q:q + 1, h * 512:(h + 1) * 512],
                                start=True, stop=True, tile_position=(q, 0))
                        xb = spool.tile([P, K], mybir.dt.float32, tag="xb")
                        nc.scalar.copy(xb[:], xps[:])
                        for blk in range(NB):
                            nc.vector._custom_dve(
                                MAM,
                                out=ctT[blk][:, m:m + 1].broadcast_to([P, K]),
                                in0=wblk[blk][:],
                                in1=xb[:],
                                s0=FMAX,
                                s1=bias_pb[:, blk:blk + 1],
                            )

            # --- endgame: transpose C^T back to natural layout ---------------
            for blk in range(NB):
                for t in range(NT):
                    tp = ppool.tile([P, P], mybir.dt.float32, tag="pe4")
                    nc.tensor.transpose(tp[:], ctT[blk][:, t * P:(t + 1) * P],
                                        ident_t[:])
                    nc.scalar.copy(ct[t][:, blk * P:(blk + 1) * P], tp[:])
            for t in range(NT):
                nc.sync.dma_start(out=out.ap()[t * P:(t + 1) * P, :],
                                  in_=ct[t][:])
    nc.compile()
    return nc


def _get_runner(replicas: int = 1):
    key = ("runner", replicas)
    if key not in _STATE:
        import jax
        import numpy as _np
        from jax.sharding import Mesh, PartitionSpec
        from jax.experimental.shard_map import shard_map
        import concourse.mybir as mybir
        from concourse import bass2jax
        from concourse.bass2jax import _bass_exec_p, install_neuronx_cc_hook

        install_neuronx_cc_hook()
        nc = build_nc(replicas)

        partition_name = (nc.partition_id_tensor.name
                          if nc.partition_id_tensor else None)
        in_names, out_names, out_avals, zero_shapes = [], [], [], []
        for alloc in nc.m.functions[0].allocations:
            if not isinstance(alloc, mybir.MemoryLocationSet):
                continue
            nm = alloc.memorylocations[0].name
            if alloc.kind == "ExternalInput":
                if nm != partition_name:
                    in_names.append(nm)
            elif alloc.kind == "ExternalOutput":
                out_names.append(nm)
                shape = tuple(alloc.tensor_shape)
                dtype = mybir.dt.np(alloc.dtype)
                out_avals.append(jax.core.ShapedArray(shape, dtype))
                zero_shapes.append((shape, dtype))
        all_in_names = list(in_names) + out_names
        if partition_name is not None:
            all_in_names.append(partition_name)

        def _body(*args):
            operands = list(args)
            if partition_name is not None:
                operands.append(bass2jax.partition_id_tensor())
            outs = _bass_exec_p.bind(
                *operands,
                out_avals=tuple(out_avals),
                in_names=tuple(all_in_names),
                out_names=tuple(out_names),
                lowering_input_output_aliases=(),
                sim_require_finite=True,
                sim_require_nnan=True,
                nc=nc,
            )
            return tuple(outs)

        devices = jax.devices()[:N_CORES]
        mesh = Mesh(_np.asarray(devices), ("core",))
        n_io = len(in_names) + len(out_names)
        fn = jax.jit(
            shard_map(_body, mesh=mesh,
                      in_specs=(PartitionSpec("core"),) * n_io,
                      out_specs=(PartitionSpec("core"),) * len(out_names),
                      check_rep=False),
            keep_unused=True,
        )
        _STATE[key] = (fn, in_names, out_names, out_avals, zero_shapes, mesh)
    return _STATE[key]


def _prepare(x, weight, bias, replicas=1):
    """device_put sharded inputs; returns a zero-arg callable running the
    kernel on device plus metadata for decoding outputs."""
    import jax
    from jax.sharding import NamedSharding, PartitionSpec
    fn, in_names, out_names, out_avals, zero_shapes, mesh = _get_runner(replicas)
    eye = np.eye(P, dtype=np.float32)
    bias_pb = np.ascontiguousarray(bias.reshape(NB, P).T)  # [128, 8]
    per_core = {
        "x": [x[c * M_LOC:(c + 1) * M_LOC] for c in range(N_CORES)],
        "weight": [weight] * N_CORES,
        "bias": [bias_pb] * N_CORES,
        "ident": [eye] * N_CORES,
    }
    concat_in = [np.concatenate(per_core[nm], axis=0) for nm in in_names]
    concat_zeros = [np.zeros((N_CORES * s[0], *s[1:]), d)
                    for (s, d) in zero_shapes]
    sharding = NamedSharding(mesh, PartitionSpec("core"))
    dev_in = [jax.device_put(a, sharding) for a in concat_in]
    dev_zero = [jax.device_put(a, sharding) for a in concat_zeros]

    def run():
        outs = fn(*dev_in, *dev_zero)
        jax.block_until_ready(outs)
        return outs

    return run, out_names, out_avals


def _run_sharded(x, weight, bias, replicas=1):
    run, out_names, out_avals = _prepare(x, weight, bias, replicas)
    return run(), out_names, out_avals


def kernel(x: np.ndarray, weight: np.ndarray, bias: np.ndarray) -> np.ndarray:
    x = np.ascontiguousarray(np.asarray(x, dtype=np.float32))
    weight = np.ascontiguousarray(np.asarray(weight, dtype=np.float32))
    bias = np.ascontiguousarray(np.asarray(bias, dtype=np.float32))
    outs, out_names, out_avals = _run_sharded(x, weight, bias, replicas=1)
    i = out_names.index("out")
    full = np.asarray(outs[i]).reshape(N_CORES * M_LOC, N)
    return full


# revision 18
# speedup vs baseline: 2.7036x; 1.0565x over previous
"""MAMDense Trainium2 kernel.

C[m, n] = max_k(x[m,k] * W[n,k]) + min_k(x[m,k] * W[n,k]) + bias[n]
x: [2048, 1024] f32, W: [1024, 1024] f32, bias: [1024] f32 -> C: [2048, 1024] f32

Strategy (data parallel over M, 8 cores, 256 rows each):
- W blocks live naturally in SBUF ([128 n-rows, K] x 8 blocks) — the custom
  DVE op reads them directly, no W movement at all.
- Each x row m is broadcast across all 128 partitions via PE (row-extract
  matmul into a PSUM quadrant, ACT copy to SBUF, ones-matmul broadcast).
  Rows are extracted 4 at a time into PSUM partition quadrants {0,32,64,96}
  (PE tile_position), so one ACT copy serves 4 rows.
- One fused custom DVE instruction per (m, n-block) computes
  running_max(w*x) + running_min(w*x) + bias over K in a single pass; the
  output AP has free-stride 0 so the final (k=K-1) value - the answer -
  lands directly in C^T[n-block][:, m]. No reduce/extract instructions.
- C^T tiles are PE-transposed back to natural [M, N] layout at the end.
"""
import numpy as np

M, K, N = 2048, 1024, 1024
N_CORES = 8
M_LOC = M // N_CORES  # 256
P = 128
NB = N // P           # 8 n-blocks
NT = M_LOC // P       # 2 m-tiles
FMAX = 3.4028234663852886e38

_STATE = {}


def _register_mam_op():
    """Register the paged multiply->scan(max)+scan(min) DVE op.

    The op streams a [P, S, Kp] 3D access pattern; a hand-built FSM state
    (mirroring the production PageIdx 3-uop machine) re-seeds both scan
    accumulators at every SUB_DIM_DONE page boundary, so one instruction
    yields S independent max+min reductions. The output AP's innermost
    stride is 0, so each page's final running value lands compactly."""
    import dataclasses
    import concourse.dve_ops as dve_ops
    from concourse.dve_ops import DveOp
    import concourse.dve_spec as dsp
    from concourse.dve_spec import Spec, Src0, Src1, C0, scan, AluOp, Trigger
    from concourse.dve_uop import DveOpSpec

    name = "MAM_PAGED_ANT"
    for op in dve_ops.OPS:
        if op.name == name:
            return op

    prod = Src0 * Src1
    spec = Spec(
        body=scan(AluOp.MAX, prod) + scan(AluOp.MIN, prod, init=C0),
        reference=lambda in0, in1, s0, s1, imm2: (
            np.maximum.accumulate(in0 * in1, axis=-1)
            + np.minimum.accumulate(in0 * in1, axis=-1)
        ),
    )

    def lower_paged(ver):
        dsp._validate_body(spec, ver)
        spec2 = dsp._hoist_stream_invariant_ops(spec)
        scans = dsp._collect(spec2.body, dsp.Scan)
        latches = dsp._collect(spec2.body, dsp.Latch)
        assert not latches
        p = dsp._build_placement(spec2, scans, dsp.N_STAGES[ver],
                                 dsp.N_LANES[ver])
        states = dsp._build_state_machine(spec2, scans, latches, p)
        assert len(states) == 2, f"expected [seed, steady], got {len(states)}"
        seed, steady = states
        steady_idx, step_idx = 1, 2
        new_steady = dataclasses.replace(
            steady,
            trigger=(Trigger.SRC_TENSOR_DONE, Trigger.SUB_DIM_DONE,
                     Trigger.NONE),
            next=(0, step_idx, 0),
        )
        step = dataclasses.replace(
            seed,
            trigger=(Trigger.SRC_TENSOR_DONE, Trigger.SUB_DIM_DONE,
                     Trigger.COUNT),
            next=(0, step_idx, steady_idx),
            repeat=1,
        )
        uops = [dsp._assemble(s) for s in (seed, new_steady, step)]
        for u in uops:
            u.validate(ver)
        return uops

    row = dve_ops._CUSTOM_DVE_ROW_BASE + len(dve_ops.OPS)
    shas, compiled = {}, {}
    for ver in ("v3", "v4"):
        try:
            s = DveOpSpec(name=name, opcode=row, uops=lower_paged(ver),
                          rd1_en=True)
            compiled[ver] = s
            shas[ver] = s.sha(ver)
        except Exception:
            pass
    op = DveOp(name, spec, subdim=True, uops_sha=shas)
    dve_ops.OPS.append(op)
    dve_ops._SUB_OPCODE_FOR_NAME[name] = row
    dve_ops.CUSTOM_DVE_SPECS[name] = spec
    # seed the compile cache with the hand-built programs (DveOp.compile
    # would otherwise re-lower the spec without the step state)
    for ver, s in compiled.items():
        dve_ops._COMPILE_CACHE[(name, ver)] = s
    return op


def build_nc(replicas: int = 1):
    """Build + compile the per-core Bacc program. `replicas` repeats the
    compute body (for timing-by-differencing in test harnesses)."""
    import concourse.bacc as bacc
    import concourse.mybir as mybir
    from concourse.tile import TileContext

    MAM = _register_mam_op()

    nc = bacc.Bacc("TRN2", target_bir_lowering=False, debug=False)
    x = nc.dram_tensor("x", [M_LOC, K], mybir.dt.float32, kind="ExternalInput")
    w = nc.dram_tensor("weight", [N, K], mybir.dt.float32, kind="ExternalInput")
    b = nc.dram_tensor("bias", [P, NB], mybir.dt.float32, kind="ExternalInput")
    ident = nc.dram_tensor("ident", [P, P], mybir.dt.float32,
                           kind="ExternalInput")
    out = nc.dram_tensor("out", [M_LOC, N], mybir.dt.float32,
                         kind="ExternalOutput")

    with TileContext(nc) as tc:
        with tc.tile_pool(name="const", bufs=1) as cpool, \
             tc.tile_pool(name="psum", bufs=2, space="PSUM") as ppool, \
             tc.tile_pool(name="stage", bufs=3) as spool:
            # --- loads -----------------------------------------------------
            xt = []
            for t in range(NT):
                xti = cpool.tile([P, K], mybir.dt.float32, name=f"xt{t}",
                                 tag=f"xt{t}")
                nc.sync.dma_start(out=xti[:], in_=x.ap()[t * P:(t + 1) * P, :])
                xt.append(xti)
            w_all = cpool.tile([P, NB * K], mybir.dt.float32, tag="w_all")
            for blk in range(NB):
                nc.sync.dma_start(out=w_all[:, blk * K:(blk + 1) * K],
                                  in_=w.ap()[blk * P:(blk + 1) * P, :])
            bias_pb = cpool.tile([P, NB], mybir.dt.float32, tag="bias_pb")
            nc.sync.dma_start(out=bias_pb[:], in_=b.ap()[:, :])
            ident_t = cpool.tile([P, P], mybir.dt.float32, tag="ident_t")
            nc.sync.dma_start(out=ident_t[:], in_=ident.ap()[:, :])
            ones_all = cpool.tile([P, P], mybir.dt.float32, tag="ones_all")
            nc.gpsimd.memset(ones_all[:], 1.0)

            ctT = cpool.tile([P, NB * M_LOC], mybir.dt.float32, tag="ctT")
            ct = [cpool.tile([P, N], mybir.dt.float32, name=f"ct{t}",
                             tag=f"ct{t}") for t in range(NT)]
            w3d = w_all[:].rearrange("p (s n) -> p s n", s=NB)
            ctT3d = ctT[:].rearrange("p (s m) -> p s m", s=NB)

            # --- main loop: groups of 4 consecutive x rows -------------------
            for _ in range(replicas):
                for g in range(M_LOC // 4):
                    pe4 = ppool.tile([P, K], mybir.dt.float32, tag="pe4")
                    for j in range(4):
                        m = 4 * g + j
                        t, r = divmod(m, P)
                        q = 32 * j
                        sel = ident_t[:, r:r + 1]
                        for h in range(K // 512):
                            nc.tensor.matmul(
                                pe4[q:q + 1, h * 512:(h + 1) * 512], sel,
                                xt[t][:, h * 512:(h + 1) * 512],
                                start=True, stop=True, tile_position=(0, q))
                    st4 = spool.tile([P, K], mybir.dt.float32, tag="st4")
                    nc.scalar.copy(st4[:], pe4[:])
                    for j in range(4):
                        m = 4 * g + j
                        q = 32 * j
                        xps = ppool.tile([P, K], mybir.dt.float32, tag="xps")
                        for h in range(K // 512):
                            nc.tensor.matmul(
                                xps[:, h * 512:(h + 1) * 512],
                                ones_all[q:q + 1, :],
                                st4[q:q + 1, h * 512:(h + 1) * 512],
                                start=True, stop=True, tile_position=(q, 0))
                        xb = spool.tile([P, K], mybir.dt.float32, tag="xb")
                        nc.scalar.copy(xb[:], xps[:])
                        nc.vector._custom_dve(
                            MAM,
                            out=ctT3d[:, :, m:m + 1].broadcast_to([P, NB, K]),
                            in0=w3d,
                            in1=xb[:].unsqueeze(1).broadcast_to([P, NB, K]),
                            s0=FMAX,
                        )

            # --- endgame: add bias, transpose C^T back to natural layout -----
            for blk in range(NB):
                nc.vector.tensor_scalar_add(
                    ctT[:, blk * M_LOC:(blk + 1) * M_LOC],
                    ctT[:, blk * M_LOC:(blk + 1) * M_LOC],
                    bias_pb[:, blk:blk + 1])
            for blk in range(NB):
                for t in range(NT):
                    tp = ppool.tile([P, P], mybir.dt.float32, tag="pe4")
                    nc.tensor.transpose(
                        tp[:], ctT[:, blk * M_LOC + t * P:blk * M_LOC + (t + 1) * P],
                        ident_t[:])
                    nc.scalar.copy(ct[t][:, blk * P:(blk + 1) * P], tp[:])
            for t in range(NT):
                nc.sync.dma_start(out=out.ap()[t * P:(t + 1) * P, :],
                                  in_=ct[t][:])
    nc.compile()
    return nc


def _get_runner(replicas: int = 1):
    key = ("runner", replicas)
    if key not in _STATE:
        import jax
        import numpy as _np
        from jax.sharding import Mesh, PartitionSpec
        from jax.experimental.shard_map import shard_map
        import concourse.mybir as mybir
        from concourse import bass2jax
        from concourse.bass2jax import _bass_exec_p, install_neuronx_cc_hook

        install_neuronx_cc_hook()
        nc = build_nc(replicas)

        partition_name = (nc.partition_id_tensor.name
                          if nc.partition_id_tensor else None)
        in_names, out_names, out_avals, zero_shapes = [], [], [], []
        for alloc in nc.m.functions[0].allocations:
            if not isinstance(alloc, mybir.MemoryLocationSet):
                continue
            nm = alloc.memorylocations[0].name
            if alloc.kind == "ExternalInput":
                if nm != partition_name:
                    in_names.append(nm)
            elif alloc.kind == "ExternalOutput":
                out_names.append(nm)
                shape = tuple(alloc.tensor_shape)
                dtype = mybir.dt.np(alloc.dtype)
                out_avals.append(jax.core.ShapedArray(shape, dtype))
                zero_shapes.append((shape, dtype))
        all_in_names = list(in_names) + out_names
        if partition_name is not None:
            all_in_names.append(partition_name)

        def _body(*args):
            operands = list(args)
            if partition_name is not None:
                operands.append(bass2jax.partition_id_tensor())
            outs = _bass_exec_p.bind(
                *operands,
                out_avals=tuple(out_avals),
                in_names=tuple(all_in_names),
                out_names=tuple(out_names),
                lowering_input_output_aliases=(),
                sim_require_finite=True,
                sim_require_nnan=True,
                nc=nc,
            )
            return tuple(outs)

        devices = jax.devices()[:N_CORES]
        mesh = Mesh(_np.asarray(devices), ("core",))
        n_io = len(in_names) + len(out_names)
        fn = jax.jit(
            shard_map(_body, mesh=mesh,
                      in_specs=(PartitionSpec("core"),) * n_io,
                      out_specs=(PartitionSpec("core"),) * len(out_names),
                      check_rep=False),
            keep_unused=True,
        )
        _STATE[key] = (fn, in_names, out_names, out_avals, zero_shapes, mesh)
    return _STATE[key]


def _prepare(x, weight, bias, replicas=1):
    """device_put sharded inputs; returns a zero-arg callable running the
    kernel on device plus metadata for decoding outputs."""
    import jax
    from jax.sharding import NamedSharding, PartitionSpec
    fn, in_names, out_names, out_avals, zero_shapes, mesh = _get_runner(replicas)
    eye = np.eye(P, dtype=np.float32)
    bias_pb = np.ascontiguousarray(bias.reshape(NB, P).T)  # [128, 8]
    per_core = {
        "x": [x[c * M_LOC:(c + 1) * M_LOC] for c in range(N_CORES)],
        "weight": [weight] * N_CORES,
        "bias": [bias_pb] * N_CORES,
        "ident": [eye] * N_CORES,
    }
    concat_in = [np.concatenate(per_core[nm], axis=0) for nm in in_names]
    concat_zeros = [np.zeros((N_CORES * s[0], *s[1:]), d)
                    for (s, d) in zero_shapes]
    sharding = NamedSharding(mesh, PartitionSpec("core"))
    dev_in = [jax.device_put(a, sharding) for a in concat_in]
    dev_zero = [jax.device_put(a, sharding) for a in concat_zeros]

    def run():
        outs = fn(*dev_in, *dev_zero)
        jax.block_until_ready(outs)
        return outs

    return run, out_names, out_avals


def _run_sharded(x, weight, bias, replicas=1):
    run, out_names, out_avals = _prepare(x, weight, bias, replicas)
    return run(), out_names, out_avals


def kernel(x: np.ndarray, weight: np.ndarray, bias: np.ndarray) -> np.ndarray:
    x = np.ascontiguousarray(np.asarray(x, dtype=np.float32))
    weight = np.ascontiguousarray(np.asarray(weight, dtype=np.float32))
    bias = np.ascontiguousarray(np.asarray(bias, dtype=np.float32))
    outs, out_names, out_avals = _run_sharded(x, weight, bias, replicas=1)
    i = out_names.index("out")
    full = np.asarray(outs[i]).reshape(N_CORES * M_LOC, N)
    return full


# revision 19
# speedup vs baseline: 2.8894x; 1.0687x over previous
"""MAMDense Trainium2 kernel.

C[m, n] = max_k(x[m,k] * W[n,k]) + min_k(x[m,k] * W[n,k]) + bias[n]
x: [2048, 1024] f32, W: [1024, 1024] f32, bias: [1024] f32 -> C: [2048, 1024] f32

Strategy (data parallel over M, 8 cores, 256 rows each):
- W blocks live naturally in SBUF ([128 n-rows, K] x 8 blocks) — the custom
  DVE op reads them directly, no W movement at all.
- Each x row m is broadcast across all 128 partitions via PE (row-extract
  matmul into a PSUM quadrant, ACT copy to SBUF, ones-matmul broadcast).
  Rows are extracted 4 at a time into PSUM partition quadrants {0,32,64,96}
  (PE tile_position), so one ACT copy serves 4 rows.
- One fused custom DVE instruction per (m, n-block) computes
  running_max(w*x) + running_min(w*x) + bias over K in a single pass; the
  output AP has free-stride 0 so the final (k=K-1) value - the answer -
  lands directly in C^T[n-block][:, m]. No reduce/extract instructions.
- C^T tiles are PE-transposed back to natural [M, N] layout at the end.
"""
import numpy as np

M, K, N = 2048, 1024, 1024
N_CORES = 8
M_LOC = M // N_CORES  # 256
P = 128
NB = N // P           # 8 n-blocks
NT = M_LOC // P       # 2 m-tiles
FMAX = 3.4028234663852886e38

_STATE = {}


def _register_mam_op():
    """Register the paged multiply->scan(max)+scan(min) DVE op.

    The op streams a [P, S, Kp] 3D access pattern; a hand-built FSM state
    (mirroring the production PageIdx 3-uop machine) re-seeds both scan
    accumulators at every SUB_DIM_DONE page boundary, so one instruction
    yields S independent max+min reductions. The output AP's innermost
    stride is 0, so each page's final running value lands compactly."""
    import dataclasses
    import concourse.dve_ops as dve_ops
    from concourse.dve_ops import DveOp
    import concourse.dve_spec as dsp
    from concourse.dve_spec import Spec, Src0, Src1, C0, scan, AluOp, Trigger
    from concourse.dve_uop import DveOpSpec

    name = "MAM_PAGED_ANT"
    for op in dve_ops.OPS:
        if op.name == name:
            return op

    prod = Src0 * Src1
    spec = Spec(
        body=scan(AluOp.MAX, prod) + scan(AluOp.MIN, prod, init=C0),
        reference=lambda in0, in1, s0, s1, imm2: (
            np.maximum.accumulate(in0 * in1, axis=-1)
            + np.minimum.accumulate(in0 * in1, axis=-1)
        ),
    )

    def lower_paged(ver):
        dsp._validate_body(spec, ver)
        spec2 = dsp._hoist_stream_invariant_ops(spec)
        scans = dsp._collect(spec2.body, dsp.Scan)
        latches = dsp._collect(spec2.body, dsp.Latch)
        assert not latches
        p = dsp._build_placement(spec2, scans, dsp.N_STAGES[ver],
                                 dsp.N_LANES[ver])
        states = dsp._build_state_machine(spec2, scans, latches, p)
        assert len(states) == 2, f"expected [seed, steady], got {len(states)}"
        seed, steady = states
        steady_idx, step_idx = 1, 2
        new_steady = dataclasses.replace(
            steady,
            trigger=(Trigger.SRC_TENSOR_DONE, Trigger.SUB_DIM_DONE,
                     Trigger.NONE),
            next=(0, step_idx, 0),
        )
        step = dataclasses.replace(
            seed,
            trigger=(Trigger.SRC_TENSOR_DONE, Trigger.SUB_DIM_DONE,
                     Trigger.COUNT),
            next=(0, step_idx, steady_idx),
            repeat=1,
        )
        uops = [dsp._assemble(s) for s in (seed, new_steady, step)]
        for u in uops:
            u.validate(ver)
        return uops

    row = dve_ops._CUSTOM_DVE_ROW_BASE + len(dve_ops.OPS)
    shas, compiled = {}, {}
    for ver in ("v3", "v4"):
        try:
            s = DveOpSpec(name=name, opcode=row, uops=lower_paged(ver),
                          rd1_en=True)
            compiled[ver] = s
            shas[ver] = s.sha(ver)
        except Exception:
            pass
    op = DveOp(name, spec, subdim=True, uops_sha=shas)
    dve_ops.OPS.append(op)
    dve_ops._SUB_OPCODE_FOR_NAME[name] = row
    dve_ops.CUSTOM_DVE_SPECS[name] = spec
    # seed the compile cache with the hand-built programs (DveOp.compile
    # would otherwise re-lower the spec without the step state)
    for ver, s in compiled.items():
        dve_ops._COMPILE_CACHE[(name, ver)] = s
    return op


def build_nc(replicas: int = 1):
    """Build + compile the per-core Bacc program. `replicas` repeats the
    compute body (for timing-by-differencing in test harnesses)."""
    import concourse.bacc as bacc
    import concourse.mybir as mybir
    from concourse.tile import TileContext

    MAM = _register_mam_op()

    nc = bacc.Bacc("TRN2", target_bir_lowering=False, debug=False)
    x = nc.dram_tensor("x", [M_LOC, K], mybir.dt.float32, kind="ExternalInput")
    w = nc.dram_tensor("weight", [N, K], mybir.dt.float32, kind="ExternalInput")
    b = nc.dram_tensor("bias", [P, NB], mybir.dt.float32, kind="ExternalInput")
    ident = nc.dram_tensor("ident", [P, P], mybir.dt.float32,
                           kind="ExternalInput")
    out = nc.dram_tensor("out", [M_LOC, N], mybir.dt.float32,
                         kind="ExternalOutput")

    with TileContext(nc) as tc:
        with tc.tile_pool(name="const", bufs=1) as cpool, \
             tc.tile_pool(name="psum", bufs=2, space="PSUM") as ppool, \
             tc.tile_pool(name="stage", bufs=3) as spool:
            # --- loads -----------------------------------------------------
            xt = []
            for t in range(NT):
                xti = cpool.tile([P, K], mybir.dt.float32, name=f"xt{t}",
                                 tag=f"xt{t}")
                nc.sync.dma_start(out=xti[:], in_=x.ap()[t * P:(t + 1) * P, :])
                xt.append(xti)
            w_all = cpool.tile([P, NB * K], mybir.dt.float32, tag="w_all")
            for blk in range(NB):
                nc.sync.dma_start(out=w_all[:, blk * K:(blk + 1) * K],
                                  in_=w.ap()[blk * P:(blk + 1) * P, :])
            bias_pb = cpool.tile([P, NB], mybir.dt.float32, tag="bias_pb")
            nc.sync.dma_start(out=bias_pb[:], in_=b.ap()[:, :])
            ident_t = cpool.tile([P, P], mybir.dt.float32, tag="ident_t")
            nc.sync.dma_start(out=ident_t[:], in_=ident.ap()[:, :])
            ones_all = cpool.tile([P, P], mybir.dt.float32, tag="ones_all")
            nc.gpsimd.memset(ones_all[:], 1.0)

            ctT = cpool.tile([P, NB * M_LOC], mybir.dt.float32, tag="ctT")
            ct = [cpool.tile([P, N], mybir.dt.float32, name=f"ct{t}",
                             tag=f"ct{t}") for t in range(NT)]
            w3d = w_all[:].rearrange("p (s n) -> p s n", s=NB)
            ctT3d = ctT[:].rearrange("p (s m) -> p s m", s=NB)

            # --- main loop: groups of 4 consecutive x rows -------------------
            for _ in range(replicas):
                for g in range(M_LOC // 4):
                    pe4 = ppool.tile([P, K], mybir.dt.float32, tag="pe4")
                    for j in range(4):
                        m = 4 * g + j
                        t, r = divmod(m, P)
                        q = 32 * j
                        sel = ident_t[:, r:r + 1]
                        for h in range(K // 512):
                            nc.tensor.matmul(
                                pe4[q:q + 1, h * 512:(h + 1) * 512], sel,
                                xt[t][:, h * 512:(h + 1) * 512],
                                start=True, stop=True, tile_position=(0, q))
                    st4 = spool.tile([P, K], mybir.dt.float32, tag="st4")
                    nc.scalar.copy(st4[:], pe4[:])
                    for j in range(4):
                        m = 4 * g + j
                        q = 32 * j
                        xps = ppool.tile([P, K], mybir.dt.float32, tag="xps")
                        for h in range(K // 512):
                            nc.tensor.matmul(
                                xps[:, h * 512:(h + 1) * 512],
                                ones_all[q:q + 1, :],
                                st4[q:q + 1, h * 512:(h + 1) * 512],
                                start=True, stop=True, tile_position=(q, 0))
                        nc.vector._custom_dve(
                            MAM,
                            out=ctT3d[:, :, m:m + 1].broadcast_to([P, NB, K]),
                            in0=w3d,
                            in1=xps[:].unsqueeze(1).broadcast_to([P, NB, K]),
                            s0=FMAX,
                        )

            # --- endgame: add bias, transpose C^T back to natural layout -----
            for blk in range(NB):
                nc.vector.tensor_scalar_add(
                    ctT[:, blk * M_LOC:(blk + 1) * M_LOC],
                    ctT[:, blk * M_LOC:(blk + 1) * M_LOC],
                    bias_pb[:, blk:blk + 1])
            for blk in range(NB):
                for t in range(NT):
                    tp = ppool.tile([P, P], mybir.dt.float32, tag="pe4")
                    nc.tensor.transpose(
                        tp[:], ctT[:, blk * M_LOC + t * P:blk * M_LOC + (t + 1) * P],
                        ident_t[:])
                    nc.scalar.copy(ct[t][:, blk * P:(blk + 1) * P], tp[:])
            for t in range(NT):
                nc.sync.dma_start(out=out.ap()[t * P:(t + 1) * P, :],
                                  in_=ct[t][:])
    nc.compile()
    return nc


def _get_runner(replicas: int = 1):
    key = ("runner", replicas)
    if key not in _STATE:
        import jax
        import numpy as _np
        from jax.sharding import Mesh, PartitionSpec
        from jax.experimental.shard_map import shard_map
        import concourse.mybir as mybir
        from concourse import bass2jax
        from concourse.bass2jax import _bass_exec_p, install_neuronx_cc_hook

        install_neuronx_cc_hook()
        nc = build_nc(replicas)

        partition_name = (nc.partition_id_tensor.name
                          if nc.partition_id_tensor else None)
        in_names, out_names, out_avals, zero_shapes = [], [], [], []
        for alloc in nc.m.functions[0].allocations:
            if not isinstance(alloc, mybir.MemoryLocationSet):
                continue
            nm = alloc.memorylocations[0].name
            if alloc.kind == "ExternalInput":
                if nm != partition_name:
                    in_names.append(nm)
            elif alloc.kind == "ExternalOutput":
                out_names.append(nm)
                shape = tuple(alloc.tensor_shape)
                dtype = mybir.dt.np(alloc.dtype)
                out_avals.append(jax.core.ShapedArray(shape, dtype))
                zero_shapes.append((shape, dtype))
        all_in_names = list(in_names) + out_names
        if partition_name is not None:
            all_in_names.append(partition_name)

        def _body(*args):
            operands = list(args)
            if partition_name is not None:
                operands.append(bass2jax.partition_id_tensor())
            outs = _bass_exec_p.bind(
                *operands,
                out_avals=tuple(out_avals),
                in_names=tuple(all_in_names),
                out_names=tuple(out_names),
                lowering_input_output_aliases=(),
                sim_require_finite=True,
                sim_require_nnan=True,
                nc=nc,
            )
            return tuple(outs)

        devices = jax.devices()[:N_CORES]
        mesh = Mesh(_np.asarray(devices), ("core",))
        n_io = len(in_names) + len(out_names)
        fn = jax.jit(
            shard_map(_body, mesh=mesh,
                      in_specs=(PartitionSpec("core"),) * n_io,
                      out_specs=(PartitionSpec("core"),) * len(out_names),
                      check_rep=False),
            keep_unused=True,
        )
        _STATE[key] = (fn, in_names, out_names, out_avals, zero_shapes, mesh)
    return _STATE[key]


def _prepare(x, weight, bias, replicas=1):
    """device_put sharded inputs; returns a zero-arg callable running the
    kernel on device plus metadata for decoding outputs."""
    import jax
    from jax.sharding import NamedSharding, PartitionSpec
    fn, in_names, out_names, out_avals, zero_shapes, mesh = _get_runner(replicas)
    eye = np.eye(P, dtype=np.float32)
    bias_pb = np.ascontiguousarray(bias.reshape(NB, P).T)  # [128, 8]
    per_core = {
        "x": [x[c * M_LOC:(c + 1) * M_LOC] for c in range(N_CORES)],
        "weight": [weight] * N_CORES,
        "bias": [bias_pb] * N_CORES,
        "ident": [eye] * N_CORES,
    }
    concat_in = [np.concatenate(per_core[nm], axis=0) for nm in in_names]
    concat_zeros = [np.zeros((N_CORES * s[0], *s[1:]), d)
                    for (s, d) in zero_shapes]
    sharding = NamedSharding(mesh, PartitionSpec("core"))
    dev_in = [jax.device_put(a, sharding) for a in concat_in]
    dev_zero = [jax.device_put(a, sharding) for a in concat_zeros]

    def run():
        outs = fn(*dev_in, *dev_zero)
        jax.block_until_ready(outs)
        return outs

    return run, out_names, out_avals


def _run_sharded(x, weight, bias, replicas=1):
    run, out_names, out_avals = _prepare(x, weight, bias, replicas)
    return run(), out_names, out_avals


def kernel(x: np.ndarray, weight: np.ndarray, bias: np.ndarray) -> np.ndarray:
    x = np.ascontiguousarray(np.asarray(x, dtype=np.float32))
    weight = np.ascontiguousarray(np.asarray(weight, dtype=np.float32))
    bias = np.ascontiguousarray(np.asarray(bias, dtype=np.float32))
    outs, out_names, out_avals = _run_sharded(x, weight, bias, replicas=1)
    i = out_names.index("out")
    full = np.asarray(outs[i]).reshape(N_CORES * M_LOC, N)
    return full
